# revision 2
# baseline (speedup 1.0000x reference)
"""Trainium2 Bass kernel for causal self-attention (B=4, T=2048, D=1024, H=16).

Sharding: 8 cores = 4 batches x 2 query-shards (identical SPMD program; role
differences live in host-fed data). Each core computes, for its batch, the
full K/V projection (prefix recompute instead of collectives), the Q
projection for its 8 query blocks of 128 rows, causal attention for all 16
heads over those query blocks, and the output projection for its rows.
Query blocks are interleaved between the two cores of a batch
({0,2,4,6,9,11,13,15} vs {1,3,5,7,8,10,12,14}) and padded to a uniform
causal-length schedule (slot j covers 2*(j+1) key chunks). Host gathers the
disjoint output slabs - no collectives.

Pipelined single-phase structure: the kernel is one loop over head pairs.
Iteration hp runs QK+exp+AV for pair hp on PE/ACT while the K/Q projections
of pair hp+1 and a slice of the V projection (VPLACE table) are interleaved
as PE filler between QK chunk groups, so the PE never drains while the ACT
engine works through the exp of the current pair. This overlaps the
projection phase (PE-bound) with the attention phase (exp/ACT-bound) that
the v1 kernel ran back to back. Engine assignment keeps queues decoupled:
ACT does exp (and tail output staging), DVE drains projection PSUM and
normalizes AV output (PE-paced), GPSIMD applies the causal masks
(ACT-paced).

AV is chunk-outer with V stationary: one ldweights of V[c,h] (65 columns,
including the ones column that accumulates the softmax denominator), then a
single wide matmul over the contiguous exp columns of every query slot the
chunk feeds, accumulating into [65,512] slot-half PSUM tiles. This replaces
the v1 kernel's 1152 N=65 matmuls (LDWEIGHTS-bound on hardware) with 384
wide ones and produces y directly in transposed layout - no 128x128
transposes. Normalization: bf16 reciprocal of the denominator row, rank-1
matmul broadcast (ones[1,128] @ rec[1,512] -> PSUM), then a DVE drain +
in-place scale (an op may read only one PSUM operand).

All matmuls run in bf16 with fp32 PSUM accumulation. QK contracts over the
64-dim head axis: even/odd head matmuls sit at partition bases 0/64 so they
land in different PE row groups and run concurrently on hardware. Matmul
PSUM writes never cross a 512-fp32 bank boundary. The output is staged to
bf16 for the final DMA; the host widens to fp32.
"""

import numpy as np
import ml_dtypes

import concourse.bass as bass
import concourse.tile as tile
from concourse import mybir
from concourse.bass_utils import run_bass_kernel_spmd

P = 128
T = 2048
D = 1024
H = 16
DH = 64
NSLOT = 8          # query blocks per core
NCH = 16           # key chunks of 128
NPAIR = H // 2
BF16 = mybir.dt.bfloat16
F32 = mybir.dt.float32

# key chunk c feeds query slots c//2..7 (uniform padded causal schedule:
# slot j covers chunks 0..2j+1)
NCOLS = [(NSLOT - c // 2) * P for c in range(NCH)]
OFF = np.cumsum([0] + NCOLS).tolist()          # exp-buffer offsets, total 9216
# chunk groups: each fits 1024 logit columns -> two [128,512] PSUM sub-tiles
CGROUPS = [[0], [1], [2], [3], [4], [5], [6], [7], [8, 9], [10, 11], [12, 13], [14, 15]]

# global query-block indices per role (core parity)
GBLOCKS = {
    0: [0, 2, 4, 6, 9, 11, 13, 15],
    1: [1, 3, 5, 7, 8, 10, 12, 14],
}

_CACHED_NC = None
LAST_RESULTS = None


def _build_nc():
    nc = bass.Bass()
    x_kvT = nc.declare_dram_parameter("x_kvT", [D, T], BF16, isOutput=False)
    x_qT = nc.declare_dram_parameter("x_qT", [D, NSLOT * P], BF16, isOutput=False)
    w_qkv = nc.declare_dram_parameter("w_qkv", [D, 3 * D], BF16, isOutput=False)
    w_out = nc.declare_dram_parameter("w_out", [D, D], BF16, isOutput=False)
    mask = nc.declare_dram_parameter("mask", [P, NCH, P], BF16, isOutput=False)
    out = nc.declare_dram_parameter("out", [NSLOT * P, D], BF16, isOutput=True)

    with tile.TileContext(nc) as tc:
        with (
            tc.tile_pool(name="persist", bufs=1) as pp,
            tc.tile_pool(name="kt_pool", bufs=3) as ktp,
            tc.tile_pool(name="qt_pool", bufs=3) as qtp,
            tc.tile_pool(name="wq_pool", bufs=2) as wqp,
            tc.tile_pool(name="wv_pool", bufs=1) as wvp,
            tc.tile_pool(name="wo_pool", bufs=1) as wop,
            tc.tile_pool(name="eh_pool", bufs=2) as ehp,
            tc.tile_pool(name="rec_pool", bufs=2) as rp,
            tc.tile_pool(name="ob_pool", bufs=2) as obp,
            tc.tile_pool(name="lpsum", bufs=3, space="PSUM") as lps,
            tc.tile_pool(name="bpsum", bufs=1, space="PSUM") as bps,
            tc.tile_pool(name="ppsum", bufs=2, space="PSUM") as pps,
            tc.tile_pool(name="spsum", bufs=2, space="PSUM") as sps,
        ):
            vA = pp.tile([P, NCH, H, DH + 1], BF16)  # V with ones column per head
            yT = pp.tile([P, 8, NSLOT * P], BF16)    # normalized attn out, transposed
            msk = pp.tile([P, NCH, P], BF16)
            xqTs = pp.tile([P, 8, NSLOT * P], BF16)
            xkTs = pp.tile([P, 8, T], BF16)
            ones_row = pp.tile([P, P], BF16)
            nc.gpsimd.memset(vA[:, :, :, DH], 1.0)
            nc.gpsimd.memset(ones_row[:], 1.0)

            kts = {}
            qts = {}
            wo = None

            # ---- projection emitters (PE work drained by DVE copies) ------
            def load_wq(et):
                wt = wqp.tile([P, 8, P], BF16, tag="wq", name=f"wq{et}")
                nc.sync.dma_start(
                    wt[:],
                    w_qkv[:, et * P:(et + 1) * P].rearrange("(dc p) e -> p dc e", p=P),
                )
                return wt

            def load_wk(et):
                wt = wqp.tile([P, 8, P], BF16, tag="wq", name=f"wk{et}")
                nc.sync.dma_start(
                    wt[:],
                    w_qkv[:, D + et * P:D + (et + 1) * P].rearrange(
                        "(dc p) e -> p dc e", p=P),
                )
                return wt

            def load_wv(nt):
                wv = wvp.tile([P, 8, 512], BF16, tag="wv", name=f"wv{nt}")
                nc.sync.dma_start(
                    wv[:],
                    w_qkv[:, 2 * D + nt * 512:2 * D + (nt + 1) * 512].rearrange(
                        "(dc p) e -> p dc e", p=P),
                )
                return wv

            def q_proj_mm(et, wt, nt):
                """One [128,512] Q^T output tile: 8 accumulating matmuls."""
                ps = pps.tile([P, 512], F32, tag="ps")
                for dc in range(8):
                    nc.tensor.matmul(
                        ps[:], wt[:, dc, :], xqTs[:, dc, nt * 512:(nt + 1) * 512],
                        start=(dc == 0), stop=(dc == 7),
                    )
                nc.vector.tensor_copy(
                    out=qts[et][:, nt * 512:(nt + 1) * 512], in_=ps[:])

            def k_proj_mm(et, wt, nt):
                ps = pps.tile([P, 512], F32, tag="ps")
                for dc in range(8):
                    nc.tensor.matmul(
                        ps[:], wt[:, dc, :], xkTs[:, dc, nt * 512:(nt + 1) * 512],
                        start=(dc == 0), stop=(dc == 7),
                    )
                nc.vector.tensor_copy(
                    out=kts[et][:, nt * 512:(nt + 1) * 512], in_=ps[:])

            def v_proj_mm(nt, wv, tt, o=0, ew=512):
                """V^T slice: e-columns [nt*512+o, +ew) for key chunk tt."""
                ps = pps.tile([P, ew], F32, tag="ps", name=f"vps{nt}_{tt}_{o}")
                for dc in range(8):
                    nc.tensor.matmul(
                        ps[:], xkTs[:, dc, tt * P:(tt + 1) * P],
                        wv[:, dc, o:o + ew],
                        start=(dc == 0), stop=(dc == 7),
                    )
                h0 = nt * 8 + o // DH
                nc.vector.tensor_copy(
                    out=vA[:, tt, h0:h0 + ew // DH, 0:DH],
                    in_=ps.rearrange("p (h d) -> p h d", d=DH),
                )

            def emit_proj_pair(et):
                """Filler list (closures) projecting K^T and Q^T for pair et."""
                kts[et] = ktp.tile([P, T], BF16, tag="kt", name=f"kt{et}")
                qts[et] = qtp.tile([P, NSLOT * P], BF16, tag="qt", name=f"qt{et}")
                wq = load_wq(et)
                wk = load_wk(et)
                items = []
                for nt in range(4):
                    items.append(lambda nt=nt: k_proj_mm(et, wk, nt))
                for nt in range(2):
                    items.append(lambda nt=nt: q_proj_mm(et, wq, nt))
                return items

            # ---- attention emitters --------------------------------------
            def emit_qk_group(hp, ehs, grp):
                """One chunk group of QK + exp + mask for head pair hp.

                Logits land in [128,512] PSUM sub-tiles (bufs=4) so the exp
                of the first half releases its bank while the second half is
                still accumulating - the next pair's QK can recycle banks
                without waiting out a full 1024-col group."""
                kt = kts[hp]
                qt = qts[hp]
                gcols = sum(NCOLS[c] for c in grp)
                for r0, eh in ((0, ehs[0]), (64, ehs[1])):
                    # chunk-column ranges packed densely at OFF[grp[0]]
                    spans = []
                    pos = 0
                    for c in grp:
                        jm = c // 2
                        ncols = NCOLS[c]
                        s = 0
                        while s < ncols:
                            w_ = min(512, ncols - s, 512 - (pos + s) % 512)
                            spans.append((pos + s, c, jm * P + s, w_))
                            s += w_
                        pos += ncols
                    base = OFF[grp[0]]
                    for sub in range(0, gcols, 512):
                        sw = min(512, gcols - sub)
                        lp = lps.tile([P, 512], F32, tag="lp")
                        for ps_, c, qs, w_ in spans:
                            if ps_ >= sub and ps_ < sub + sw:
                                nc.tensor.matmul(
                                    lp[:, ps_ - sub:ps_ - sub + w_],
                                    kt[r0:r0 + 64, c * P:(c + 1) * P],
                                    qt[r0:r0 + 64, qs:qs + w_],
                                    start=True, stop=True,
                                )
                        nc.scalar.activation(
                            eh[:, base + sub:base + sub + sw], lp[:, :sw],
                            mybir.ActivationFunctionType.Exp, scale=0.125,
                        )
                    for c in grp:
                        # causal/padding mask on the diagonal slot of chunk c
                        nc.gpsimd.tensor_mul(
                            out=eh[:, OFF[c]:OFF[c] + P],
                            in0=eh[:, OFF[c]:OFF[c] + P],
                            in1=msk[:, c, :],
                        )

            def emit_out_slot(j):
                for nh in range(2):
                    op = pps.tile([P, 512], F32, tag="ps", name=f"op{j}_{nh}")
                    for kc in range(8):
                        nc.tensor.matmul(
                            op[:],
                            yT[:, kc, j * P:(j + 1) * P],
                            wo[:, kc, nh * 512:(nh + 1) * 512],
                            start=(kc == 0), stop=(kc == 7),
                        )
                    ob = obp.tile([P, 512], BF16, tag="ob")
                    # ACT is exp-free during the output projections; staging
                    # there keeps DVE clear for the last norms
                    nc.scalar.copy(out=ob[:], in_=op[:])
                    nc.sync.dma_start(out[j * P:(j + 1) * P, nh * 512:(nh + 1) * 512],
                                      ob[:])

            def new_ya(hp, parity):
                """Two [65,512] PSUM tiles per head: slots 0-3 and 4-7.
                V is the stationary operand; rows 0-63 collect y, row 64
                (the ones column of vA) collects the softmax denominator."""
                return [
                    sps.tile([DH + 1, 512], F32, tag="ya",
                             name=f"ya{hp}_{parity}_{half}")
                    for half in range(2)
                ]

            def emit_av_chunk(hp, eh, parity, ya, c):
                """V-stationary AV matmuls of chunk c into both slot halves:
                one ldweights of V[c,h] (65 cols), wide moving exp columns."""
                h = 2 * hp + parity
                jm = c // 2
                if c < 8:
                    # slots jm..3 -> ya[0] cols jm*128..512
                    nc.tensor.matmul(
                        ya[0][:, jm * P:512],
                        vA[:, c, h, :],
                        eh[:, OFF[c]:OFF[c] + (4 - jm) * P],
                        start=(c == 0), stop=(c == 7),
                        skip_group_check=True,
                    )
                js = max(jm, 4)
                # slots js..7 -> ya[1] cols (js-4)*128..512
                nc.tensor.matmul(
                    ya[1][:, (js - 4) * P:512],
                    vA[:, c, h, :],
                    eh[:, OFF[c] + (js - jm) * P:OFF[c] + (8 - jm) * P],
                    start=(c == 0), stop=(c == 15),
                    skip_group_check=True,
                )

            def emit_norm(hp, parity, ya, half):
                """yT[64p:64p+64, hp, half*512:...] = ya[0:64]/ya[64].

                The denominator row (PSUM partition 64) is broadcast to all
                partitions with a rank-1 matmul: ones[1,128] stationary x
                rec[1,512] moving -> [128,512] PSUM, 512 PE cycles."""
                r = rp.tile([DH + 1, 512], BF16, tag="rec")
                with nc.allow_low_precision(
                        reason="softmax denominator reciprocal, bf16 ok"):
                    nc.vector.reciprocal(r[DH:DH + 1, :], ya[half][DH:DH + 1, :])
                brec = bps.tile([P, 512], F32, tag="brc", name=f"brec{hp}_{parity}_{half}")
                nc.tensor.matmul(
                    brec[:], ones_row[DH:DH + 1, :], r[DH:DH + 1, :],
                    start=True, stop=True,
                )
                # DVE may read only one PSUM operand per op: drain ya to
                # yT first, then scale in place against the PSUM brec.
                yslice = yT[64 * parity:64 * parity + 64, hp,
                            half * 512:(half + 1) * 512]
                nc.vector.tensor_copy(out=yslice, in_=ya[half][0:DH, :])
                nc.vector.tensor_mul(out=yslice, in0=yslice, in1=brec[0:DH, :])

            # ---- prologue: DMAs ordered by first use, then Q/K of pair 0 --
            xq_r = x_qT.rearrange("(dc p) t -> p dc t", p=P)
            xk_r = x_kvT.rearrange("(dc p) t -> p dc t", p=P)
            wq0 = load_wq(0)
            for nt in range(2):
                nc.sync.dma_start(xqTs[:, :, nt * 512:(nt + 1) * 512],
                                  xq_r[:, :, nt * 512:(nt + 1) * 512])
            wk0 = load_wk(0)
            for nt in range(4):
                nc.sync.dma_start(xkTs[:, :, nt * 512:(nt + 1) * 512],
                                  xk_r[:, :, nt * 512:(nt + 1) * 512])
            wv0 = load_wv(0)
            nc.sync.dma_start(msk[:], mask[:])

            kts[0] = ktp.tile([P, T], BF16, tag="kt", name="kt0")
            qts[0] = qtp.tile([P, NSLOT * P], BF16, tag="qt", name="qt0")
            for nt in range(2):
                q_proj_mm(0, wq0, nt)
            for nt in range(4):
                k_proj_mm(0, wk0, nt)

            # V projection placement: per-iteration lists of
            # (nt, tt, e-offset, e-width) spreading the V matmuls across the
            # pipeline so every iteration (including the last) has PE filler,
            # while pair p's V columns land before iteration p's AV reads
            # them. Lists whose V feeds the CURRENT iteration's AV come
            # first in the filler so the pop schedule stays ahead of the
            # per-chunk AV trickle.
            VPLACE = {
                0: [(0, tt, 0, 256) for tt in range(NCH)],        # pairs 0,1
                1: [(0, tt, 256, 256) for tt in range(8)],        # pairs 2,3
                2: [(0, tt, 256, 256) for tt in range(8, NCH)],
                3: [(1, tt, 0, 256) for tt in range(8)],          # pairs 4,5
                4: [(1, tt, 0, 256) for tt in range(8, NCH)],
                5: [(1, tt, 256, 128) for tt in range(8)],        # pair 6
                6: [(1, tt, 256, 128) for tt in range(8, NCH)],
                7: [(1, tt, 384, 128) for tt in range(NCH)],      # pair 7
            }

            # ---- main pipeline over head pairs ---------------------------
            wvs = {0: wv0}
            for hp in range(NPAIR):
                filler = []
                for (nt, tt, o, ew) in VPLACE[hp]:
                    wv = wvs[nt]
                    filler.append(
                        lambda nt=nt, tt=tt, o=o, ew=ew, wv=wv:
                        v_proj_mm(nt, wv, tt, o, ew))
                if hp < NPAIR - 1:
                    filler.extend(emit_proj_pair(hp + 1))
                if hp == 1:
                    wvs[1] = load_wv(1)
                if hp == NPAIR - 3:
                    wo = wop.tile([P, 8, D], BF16)
                    for dc in range(8):
                        nc.sync.dma_start(wo[:, dc, :],
                                          w_out[dc * P:(dc + 1) * P, :])

                ehs = (ehp.tile([P, OFF[NCH]], BF16, tag="exph", name=f"eh{hp}a"),
                       ehp.tile([P, OFF[NCH]], BF16, tag="exph", name=f"eh{hp}b"))

                # schedule: QK group g feeds exp (ACT) and masks (Pool); the
                # even head's AV matmuls trickle right behind each group
                # (chunk-outer, V stationary, accumulating into the ya slot
                # halves), with projection filler spread between groups. The
                # odd head's AV runs as one dense block at iteration end -
                # its ya tiles reuse the even head's PSUM via the pool. On
                # the last pair the output projections follow the norms.
                last = hp == NPAIR - 1
                nfill = len(filler)
                npts = len(CGROUPS)
                fi = 0
                ya_e = new_ya(hp, 0)
                for gi, grp in enumerate(CGROUPS):
                    emit_qk_group(hp, ehs, grp)
                    want = (gi + 1) * nfill // npts
                    while fi < want:
                        filler[fi]()
                        fi += 1
                    for c in grp:
                        emit_av_chunk(hp, ehs[0], 0, ya_e, c)
                    if gi == 7:
                        emit_norm(hp, 0, ya_e, 0)
                    elif gi == 11:
                        emit_norm(hp, 0, ya_e, 1)
                while fi < nfill:
                    filler[fi]()
                    fi += 1
                # odd head: dense AV block; the c8-15 matmuls cover the
                # latency of the half-0 norm chain before the (last-pair)
                # output projections start.
                ya_o = new_ya(hp, 1)
                for c in range(8):
                    emit_av_chunk(hp, ehs[1], 1, ya_o, c)
                emit_norm(hp, 1, ya_o, 0)
                for c in range(8, NCH):
                    emit_av_chunk(hp, ehs[1], 1, ya_o, c)
                emit_norm(hp, 1, ya_o, 1)
                if last:
                    for j in range(NSLOT):
                        emit_out_slot(j)

    _split_waits(nc, 1)
    return nc


def _split_waits(nc, maxw=1):
    """walrus rejects instructions with more than one sync wait; hoist extra
    waits onto preceding same-engine Drain instructions."""
    nsplit = 0
    for f in nc.m.functions:
        for b in f.blocks:
            insts = b.instructions
            new = []
            changed = False
            for inst in insts:
                si = inst.sync_info
                if si is not None and len(si.on_wait) > maxw:
                    waits = list(si.on_wait)
                    chunks = [waits[i:i + maxw] for i in range(0, len(waits), maxw)]
                    for ci, ch in enumerate(chunks[:-1]):
                        d = mybir.InstDrain(name=f"{inst.name}-wsplit{ci}", ins=[], outs=[])
                        d.engine = inst.engine
                        d.sync_info = mybir.SyncInfo(on_wait=ch, on_update=[])
                        new.append(d)
                        nsplit += 1
                    inst.sync_info = mybir.SyncInfo(
                        on_wait=chunks[-1], on_update=list(si.on_update))
                    changed = True
                new.append(inst)
            if changed:
                b.instructions = new
    return nsplit


def _host_mask(role):
    g = GBLOCKS[role]
    m = np.zeros((P, NCH, P), np.float32)
    for c in range(NCH):
        j = c // 2
        kk = c * P + np.arange(P)[:, None]       # global key index
        qq = g[j] * P + np.arange(P)[None, :]    # global query index
        m[:, c, :] = (kk <= qq).astype(np.float32)
    return m.astype(ml_dtypes.bfloat16)


def kernel(x, w_qkv, w_out):
    global _CACHED_NC, LAST_RESULTS
    x = np.asarray(x)
    w_qkv = np.asarray(w_qkv)
    w_out = np.asarray(w_out)
    B = x.shape[0]
    assert x.shape == (B, T, D) and B * 2 == 8

    if _CACHED_NC is None:
        _CACHED_NC = _build_nc()
    nc = _CACHED_NC

    wq_b = w_qkv.astype(ml_dtypes.bfloat16)
    wo_b = w_out.astype(ml_dtypes.bfloat16)
    masks = {r: _host_mask(r) for r in (0, 1)}

    in_maps = []
    for core in range(8):
        b, role = divmod(core, 2)
        xb = x[b].astype(ml_dtypes.bfloat16)
        g = GBLOCKS[role]
        xq = np.concatenate([xb[gi * P:(gi + 1) * P] for gi in g], axis=0)
        in_maps.append({
            "x_kvT": np.ascontiguousarray(xb.T),
            "x_qT": np.ascontiguousarray(xq.T),
            "w_qkv": wq_b,
            "w_out": wo_b,
            "mask": masks[role],
        })

    res = run_bass_kernel_spmd(nc, in_maps, core_ids=list(range(8)))
    LAST_RESULTS = res

    y = np.empty((B, T, D), np.float32)
    for core in range(8):
        b, role = divmod(core, 2)
        slab = res.results[core]["out"].astype(np.float32)
        g = GBLOCKS[role]
        for j, gi in enumerate(g):
            y[b, gi * P:(gi + 1) * P, :] = slab[j * P:(j + 1) * P, :]
    return y
